# revision 30
# baseline (speedup 1.0000x reference)
"""APD2Net (VGG-style conv net over graph ops) on 8 trn2 NeuronCores.

Sharding: 2 batches x 4 H-quarters. Channels on SBUF partitions; every conv is
fp16 matmuls (fp32 PSUM accumulate) over 3x3 taps via shifted access patterns;
ACT does ReLU+bias epilogues; DVE does 2x2 maxpools. Halos at pool2/pool3 are
exchanged with one 4-way AllGather per stage + mask-select (static SPMD
graph). Each stage is split interior-first: halo-independent rows run while
the AllGather rendezvous absorbs cross-core start skew; boundary strips run
after, as paired-strip matmuls (two row bands per PSUM bank). Out-of-image
rows on edge cores stay exactly zero via per-core zeroed bias variants
(relu(0*W + 0) = 0 matches the reference's zero padding). Conv1's im2col is
pre-shifted on the host so each superblock loads with one wide DMA.
"""
import numpy as np

import concourse.bacc as bacc
import concourse.mybir as mybir
import concourse.tile as tile
from concourse.bass_utils import run_bass_kernel_spmd

f32 = mybir.dt.float32
f32r = mybir.dt.float32r
f16 = mybir.dt.float16
AF = mybir.ActivationFunctionType
ALU = mybir.AluOpType

N_CORES = 8
G = 1   # front guard elems
G2 = 4  # back guard elems
BR = 8  # streaming block rows at 384-res


def _zero(nc, ap):
    nc.vector.memset(ap.bitcast(f32), 0.0)
    nc.vector.tensor_scalar_mul(ap, ap, 0.0)


class Buf:
    """Padded activation buffer: per group, G + R*Wp + G2 elems.
    `extra` tail slack lets strip-paired rhs APs span past the last group."""

    def __init__(self, pool, name, ngrp, R, Wp, dtype=f16, extra=0):
        self.ngrp, self.R, self.Wp = ngrp, R, Wp
        self.gstride = R * Wp + G + G2
        self.t = pool.tile([128, ngrp * self.gstride + extra], dtype, name=name)

    def span(self, g, off, n):
        base = g * self.gstride + G + off
        return self.t[:, base:base + n]

    def rows(self, g, r0, nrows, c0, ncols):
        """[128, nrows, ncols] at rows r0.., cols c0.. (uses back guard slack)."""
        base = g * self.gstride + G + r0 * self.Wp
        ap = self.t[:, base:base + nrows * self.Wp]
        ap = ap.rearrange("p (r w) -> p r w", r=nrows, w=self.Wp)
        return ap[:, :, c0:c0 + ncols]

    def zero_all(self, nc):
        _zero(nc, self.t[:])

    def zero_edges(self, nc, ncols=1, rows=()):
        """Zero only the never-written elems: group guards, pad columns,
        and explicitly listed (r0, nrows) row spans (halo rows)."""
        t3 = self.t[:, 0:self.ngrp * self.gstride] \
            .rearrange("p (g s) -> p g s", g=self.ngrp)
        nc.vector.memset(t3[:, :, 0:G], 0.0)
        nc.vector.memset(t3[:, :, G + self.R * self.Wp:], 0.0)
        for g in range(self.ngrp):
            body = t3[:, g, G:G + self.R * self.Wp]
            b3 = body.rearrange("p (r w) -> p r w", r=self.R, w=self.Wp)
            nc.vector.memset(b3[:, :, 0:ncols], 0.0)
            nc.vector.memset(b3[:, :, self.Wp - ncols:self.Wp], 0.0)
            for (r0, nr) in rows:
                nc.vector.memset(
                    t3[:, g, G + r0 * self.Wp:G + (r0 + nr) * self.Wp], 0.0)


def psum_rows(pt, r0, nrows, c0, ncols, Wp):
    ap = pt[:, r0 * Wp:(r0 + nrows) * Wp]
    ap = ap.rearrange("p (r w) -> p r w", r=nrows, w=Wp)
    return ap[:, :, c0:c0 + ncols]


def packed6_chunks():
    # B chunks padded to K=128 with zero weight rows (uniform K avoids PE
    # array reconfig stalls); upper-half rhs reads are real data times zero.
    ch = []
    for d, dy in enumerate((-1, 0, 1)):
        ch.append((128, 2 * d, dy, 0))      # A: lower dx=0, upper dx=-1
        ch.append((128, 2 * d + 1, dy, 1))  # B: lower dx=+1, upper zero-w
    return ch, [0] * 6


def plain_chunks(cin_groups):
    ch, grp = [], []
    for g in range(cin_groups):
        for dy in (-1, 0, 1):
            for dx in (-1, 0, 1):
                ch.append((128, len(ch), dy, dx))
                grp.append(g)
    return ch, grp


def conv_layer(nc, psum, src, wparts, chunks, grp, nblk, dst_write, tiles,
               name, order=None):
    """Grouped chunk-major: groups of G tiles (G*nblk<=8 psum banks); within a
    group, loop chunks outer so the same lhsT streams across all banks."""
    Wp = src.Wp
    order = list(order) if order is not None else list(range(len(tiles)))
    G = max(1, 8 // nblk)
    nmm = len(chunks)
    for g0 in range(0, len(order), G):
        gtis = order[g0:g0 + G]
        pts = {}
        for ti in gtis:
            for blk in range(nblk):
                pts[(ti, blk)] = psum.tile([128, 512], f32,
                                           name=f"ps_{name}_{ti}_{blk}", tag="ps")
        for ci, (ksz, wcol, dy, dx) in enumerate(chunks):
            wt_, lo, hi = next((w, l, h) for (w, l, h) in wparts
                               if l <= wcol < h)
            for blk in range(nblk):
                lhsT = wt_[0:ksz, wcol - lo, blk * 128:(blk + 1) * 128] \
                    if wt_.shape[2] > 128 else wt_[0:ksz, wcol - lo, :]
                for ti in gtis:
                    r0, nrows = tiles[ti]
                    N = nrows * Wp
                    off = (r0 + 1 + dy) * Wp + dx
                    rhs = src.span(grp[ci], off, N)
                    if ksz < 128:
                        rhs = rhs[0:ksz]
                    nc.tensor.matmul(pts[(ti, blk)][:, 0:N], lhsT=lhsT, rhs=rhs,
                                     start=(ci == 0), stop=(ci == nmm - 1))
        for ti in gtis:
            r0, nrows = tiles[ti]
            for blk in range(nblk):
                dst_write(blk, r0, nrows, pts[(ti, blk)], ti)


def conv_layer_cm(nc, psum, src, wparts, chunks, grp, nblk, dst_write, tiles,
                  name):
    """Chunk-major: all (tile, blk) psum tiles live; weights stream by part.
    Each tile is (r0, nrows) or a pair [(r0a, n), (r0b, n)] of row strips
    computed in ONE matmul per chunk (keeps the f32r free dim >= 256)."""
    Wp = src.Wp
    tiles = [t if isinstance(t, list) else [t] for t in tiles]
    assert len(tiles) * nblk <= 8
    pts = {}
    for ti in range(len(tiles)):
        for blk in range(nblk):
            pts[(ti, blk)] = psum.tile([128, 512], f32,
                                       name=f"ps_{name}_{ti}_{blk}", tag="ps")
    nmm = len(chunks)
    for ci, (ksz, wcol, dy, dx) in enumerate(chunks):
        wt_, lo, hi = next((w, l, h) for (w, l, h) in wparts if l <= wcol < h)
        for blk in range(nblk):
            lhsT = wt_[0:ksz, wcol - lo, blk * 128:(blk + 1) * 128] \
                if wt_.shape[2] > 128 else wt_[0:ksz, wcol - lo, :]
            for ti, strips in enumerate(tiles):
                r0, nrows = strips[0]
                N1 = nrows * Wp
                off = (r0 + 1 + dy) * Wp + dx
                if len(strips) == 1:
                    rhs = src.span(grp[ci], off, N1)
                    out = pts[(ti, blk)][:, 0:N1]
                else:
                    stride = (strips[1][0] - r0) * Wp
                    rhs = src.span(grp[ci], off, 2 * stride) \
                        .rearrange("p (s x) -> p s x", s=2)[:, :, 0:N1]
                    out = pts[(ti, blk)][:, 0:2 * N1] \
                        .rearrange("p (s x) -> p s x", s=2)
                if ksz < 128:
                    rhs = rhs[0:ksz]
                nc.tensor.matmul(out, lhsT=lhsT, rhs=rhs,
                                 start=(ci == 0), stop=(ci == nmm - 1))
    for ti, strips in enumerate(tiles):
        for blk in range(nblk):
            for si, (r0, nrows) in enumerate(strips):
                N1 = nrows * Wp
                dst_write(blk, r0, nrows,
                          pts[(ti, blk)][:, si * N1:(si + 1) * N1], ti)


def seg_rows(r0, nrows, mtop, mbot, R):
    """Split tile rows into (start, len, bias_class) segments."""
    segs = []
    for r in range(r0, r0 + nrows):
        bc = 1 if r < mtop else (2 if r >= R - mbot else 0)
        if segs and segs[-1][2] == bc:
            segs[-1][1] += 1
        else:
            segs.append([r, 1, bc])
    return segs


class _StopBuild(Exception):
    pass


def build_nc(stage=None):
    """stage: None=full, or one of 'pool1','l4','pool2','l7','pool3','l9'
    to truncate the graph and dump that buffer to the debug output."""
    nc = bacc.Bacc(None, target_bir_lowering=False)

    x_in = nc.declare_dram_parameter("x", [36, 43624], f16, isOutput=False)
    WSHAPES = {
        "w1": [36, 1, 128], "w2": [128, 6, 128], "w3": [128, 6, 128],
        "w4": [128, 9, 128], "w5": [128, 9, 256], "w6": [128, 18, 256],
        "w7": [128, 18, 256], "w8": [128, 18, 512], "w9": [128, 36, 512],
        "w10": [128, 36, 512],
    }
    wp = {k: nc.declare_dram_parameter(k, v, f16, isOutput=False)
          for k, v in WSHAPES.items()}
    NBLK = {"b1": 1, "b2": 1, "b3": 1, "b4": 1, "b5": 2, "b6": 2, "b7": 2,
            "b8": 4, "b9": 4, "b10": 4}
    bp = {k: nc.declare_dram_parameter(k, [128, n * 3], f32, isOutput=False)
          for k, n in NBLK.items()}
    masks1 = nc.declare_dram_parameter("masks1", [128, 8], f32, isOutput=False)
    rowmask = nc.declare_dram_parameter("rowmask", [128, 2], f32, isOutput=False)
    masks2 = nc.declare_dram_parameter("masks2", [128, 8], f32, isOutput=False)
    STAGES = ["pool1", "l4", "pool2", "l5", "l7", "pool3", "l9", "conv1s0"]
    SI = -1 if stage == "conv1s0" else (
        99 if stage is None else STAGES.index(stage))
    DUMPSZ = {"conv1s0": 10 * 194 + 5,
              "pool1": 52 * 194 + 5, "l4": 48 * 194 + 5, "pool2": 30 * 98 + 5,
              "l5": 2 * (28 * 98 + 5), "l7": 2 * (24 * 98 + 5),
              "pool3": 2 * (18 * 50 + 5), "l9": 4 * (14 * 50 + 5)}
    if stage is None:
        y_out = nc.declare_dram_parameter("out", [4, 128, 576], f32, isOutput=True)
    else:
        y_out = nc.declare_dram_parameter("out", [128, DUMPSZ[stage]], f32,
                                          isOutput=True)

    with tile.TileContext(nc) as tc:
        with tc.tile_pool(name="persist", bufs=1) as persist, \
             tc.tile_pool(name="dram", bufs=1, space="DRAM") as dram:
            psum = None  # main PSUM pool, opened after phase A1
            try:

                bt = {}
                for k in bp:
                    bt[k] = persist.tile(list(bp[k].shape), f32, name=f"bt_{k}")
                    nc.gpsimd.dma_start(out=bt[k][:], in_=bp[k][:])
                m1t = persist.tile([128, 8], f32, name="m1t")
                nc.gpsimd.dma_start(out=m1t[:], in_=masks1[:])
                m2t = persist.tile([128, 8], f32, name="m2t")
                nc.gpsimd.dma_start(out=m2t[:], in_=masks2[:])
                rmt = persist.tile([128, 2], f32, name="rmt")
                nc.gpsimd.dma_start(out=rmt[:], in_=rowmask[:])
                zt = persist.tile([128, 1], f32, name="zt")
                nc.vector.memset(zt[:], 0.0)

                def relu_write(out2d, in2d, bias_ap, use_dve):
                    if use_dve:
                        p0 = out2d.base_partition()
                        np_ = out2d.shape[0]
                        zb = zt[p0:p0 + np_, 0:1]
                        for _ in range(len(out2d.shape) - 2):
                            zb = zb.unsqueeze(1)
                        nc.vector.scalar_tensor_tensor(
                            out=out2d, in0=in2d, scalar=bias_ap,
                            in1=zb.to_broadcast(list(out2d.shape)),
                            op0=ALU.add, op1=ALU.max)
                    else:
                        nc.scalar.activation(out2d, in2d, AF.Relu,
                                             bias=bias_ap, scale=1.0)

                def mask_row(buf, row, side):
                    """Multiply buffer row (all groups) by per-core 0/1 mask."""
                    base = G + row * buf.Wp
                    ap = buf.t[:, 0:buf.ngrp * buf.gstride] \
                        .rearrange("p (g s) -> p g s", g=buf.ngrp)
                    ap = ap[:, :, base:base + buf.Wp]
                    nc.vector.tensor_scalar_mul(ap, ap, rmt[:, side:side + 1])

                w5t = persist.tile([128, 9, 256], f16, name="w5t")
                nc.gpsimd.dma_start(out=w5t[:], in_=wp["w5"][:])
                pool2 = Buf(persist, "pool2", 1, 30, 98)
                pool3 = Buf(persist, "pool3", 2, 18, 50, extra=512)
                pool2.zero_edges(nc, rows=((0, 3), (27, 3)))
                pool3.zero_edges(nc, rows=((0, 3), (15, 3)))

                def load_w(pool_, key, lo, hi, tagsz):
                    t = pool_.tile([WSHAPES[key][0], hi - lo, WSHAPES[key][2]], f16,
                                   name=f"wt_{key}_{lo}", tag=f"w_{key}_{lo}",
                                   bufs=1)
                    eng = nc.scalar if (lo // 9) % 2 else nc.sync
                    eng.dma_start(out=t[:], in_=wp[key][:, lo:hi, :])
                    return (t, lo, hi)

                # ======== phase A (384/192-res) ========
                with tc.tile_pool(name="wpA", bufs=1) as wpA:
                    def load_w_static(pool_, key, eng=None):
                        t = pool_.tile(list(WSHAPES[key]), f16, name=f"wts_{key}",
                                       tag=f"wts_{key}")
                        (eng or nc.scalar).dma_start(out=t[:], in_=wp[key][:])
                        return (t, 0, WSHAPES[key][1])
                    w1p = [load_w_static(wpA, "w1")]
                    w2p = [load_w_static(wpA, "w2", nc.gpsimd)]
                    w3p = [load_w_static(wpA, "w3", nc.gpsimd)]
                    w4p = [load_w_static(wpA, "w4", nc.gpsimd)]
                    pool1 = Buf(wpA, "pool1", 1, 52, 194)
                    pool1.zero_edges(nc, ncols=2)

                    # ---- A1: conv1 -> conv2 -> pool1 (pair-packed streaming) ----
                    # conv1 psum M = [64ch "odd" y1[2t-1] | 64ch "even" y1[2t]];
                    # conv2 consumes via 6 K=128 chunks at two col offsets.
                    with tc.tile_pool(name="phA1", bufs=1) as pa, \
                         tc.tile_pool(name="im2c", bufs=2) as pim, \
                         tc.tile_pool(name="psA1", bufs=1, space="PSUM") as psA1:
                        NS1 = 3
                        c1ring = [Buf(pa, f"c1s{i}", 1, BR + 2, 194)
                                  for i in range(NS1)]
                        for s in c1ring:
                            s.zero_edges(nc, ncols=1)

                        def c1_bcol(j):
                            return 1 if j < 5 else (2 if j >= 101 else 0)

                        def c1_write(slot, lr0, pc1, bk0, nrows, j0, use_dve):
                            # rows j0.. from pc1 banks bk0.., 2 rows/bank
                            segs = []
                            for k in range(nrows):
                                bc = c1_bcol(j0 + k)
                                if segs and segs[-1][2] == bc:
                                    segs[-1][1] += 1
                                else:
                                    segs.append([k, 1, bc])
                            for (k0, kn, bc) in segs:
                                # split by bank alignment (<=3D APs only)
                                k = k0
                                while k < k0 + kn:
                                    if k % 2 == 0 and k0 + kn - k >= 2:
                                        n = (k0 + kn - k) // 2 * 2
                                        src = pc1[:, bk0 + k // 2:
                                                  bk0 + (k + n) // 2, 0:386]
                                    else:
                                        n = 1
                                        bk, r = bk0 + k // 2, k % 2
                                        src = pc1[:, bk, r * 193:(r + 1) * 193]
                                    relu_write(
                                        slot.rows(0, lr0 + k, n, 0, 193),
                                        src, bt["b1"][:, bc:bc + 1], use_dve)
                                    k += n

                        def c1_fixups(slot):
                            # lower t=0 is y1[-1], upper t=192 is y1[384]: both 0
                            v = slot.rows(0, 0, BR + 2, 0, 194)
                            nc.vector.memset(v[0:64, :, 0:1], 0.0)
                            nc.vector.memset(v[64:128, :, 192:193], 0.0)

                        CH6 = [(dyi, jj) for dyi in range(3) for jj in range(2)]
                        NB2 = 13
                        for b in range(NB2 + 2):
                            if stage == "conv1s0" and b == 2:
                                nc.sync.dma_start(out=y_out[:],
                                                  in_=c1ring[0].t[:].bitcast(f32))
                                raise _StopBuild
                            r0c1 = 8 * b
                            nr1 = max(0, min(8, 106 - r0c1))
                            if nr1 > 0:
                                # im2col superblock: host pre-shifted the 36
                                # (dy, et, c) variants per partition, so one
                                # wide DMA (split across 2 queues) suffices.
                                if b % 2 == 0:
                                    rsb = 16 * (b // 2)
                                    nrs = min(16, 106 - rsb)
                                    imt = pim.tile([36, 16 * 386 + 4], f16,
                                                   name="imt", tag="im")
                                    so = rsb * 386
                                    nel = nrs * 386 + 2
                                    if b == 0:
                                        # prime rows 0-9 first: block 0's
                                        # matmuls need only those, so the
                                        # PE starts ~2us sooner
                                        pr = 10 * 386 + 2
                                        nc.sync.dma_start(
                                            out=imt[0:18, 0:pr],
                                            in_=x_in[0:18, 0:pr])
                                        nc.scalar.dma_start(
                                            out=imt[18:36, 0:pr],
                                            in_=x_in[18:36, 0:pr])
                                        nc.sync.dma_start(
                                            out=imt[0:18, 10 * 386:nel],
                                            in_=x_in[0:18, 10 * 386:nel])
                                        nc.scalar.dma_start(
                                            out=imt[18:36, 10 * 386:nel],
                                            in_=x_in[18:36, 10 * 386:nel])
                                    else:
                                        nc.sync.dma_start(
                                            out=imt[0:18, 0:nel],
                                            in_=x_in[0:18, so:so + nel])
                                        nc.scalar.dma_start(
                                            out=imt[18:36, 0:nel],
                                            in_=x_in[18:36, so:so + nel])
                                    cur_imt, cur_rsb = imt, rsb
                                npairs = (nr1 + 1) // 2
                                pc1s = {}
                                for h in range((npairs + 1) // 2):
                                    pc1 = psA1.tile([128, 2, 512], f32,
                                                    name=f"pc1_{b}_{h}",
                                                    tag="c1", bufs=2)
                                    pc1s[h] = pc1
                                    for bk in range(min(2, npairs - 2 * h)):
                                        mb = (r0c1 - cur_rsb
                                              + 4 * h + 2 * bk) * 386
                                        rhs = cur_imt[0:36, mb:mb + 772] \
                                            .rearrange("p (r w) -> p r w", r=2,
                                                       w=386)[:, :, 0:386:2]
                                        nc.tensor.matmul(
                                            pc1[:, bk, 0:386],
                                            lhsT=w1p[0][0][0:36, 0, :],
                                            rhs=rhs, start=True, stop=True)
                                # halo rows 8,9 of previous slot (= rows 8b, 8b+1)
                                if b >= 1:
                                    pslot = c1ring[(b - 1) % NS1]
                                    c1_write(pslot, 8, pc1s[0], 0, 2, r0c1,
                                             use_dve=False)
                                    if r0c1 <= 101 < r0c1 + 2:
                                        nc.vector.tensor_scalar_mul(
                                            pslot.rows(0, 8 + 101 - r0c1, 1, 0, 194),
                                            pslot.rows(0, 8 + 101 - r0c1, 1, 0, 194),
                                            rmt[:, 1:2])
                                    c1_fixups(pslot)
                                # own slot rows (b <= 12)
                                if b <= NB2 - 1:
                                    slot = c1ring[b % NS1]
                                    for h in range(2):
                                        c1_write(slot, 4 * h, pc1s[h], 0, 4,
                                                 r0c1 + 4 * h,
                                                 use_dve=(h % 2 == 1))
                                    for jm, side in ((4, 0), (101, 1)):
                                        if r0c1 <= jm < r0c1 + 8:
                                            lr = jm - r0c1
                                            nc.vector.tensor_scalar_mul(
                                                slot.rows(0, lr, 1, 0, 194),
                                                slot.rows(0, lr, 1, 0, 194),
                                                rmt[:, side:side + 1])
                            if 2 <= b <= NB2 + 1:
                                bb = b - 2
                                slot = c1ring[bb % NS1]
                                for h in range(2):
                                    i0 = 8 * bb + 4 * h
                                    pc2 = psA1.tile([128, 2, 512], f32,
                                                    name=f"pc2_{bb}_{h}",
                                                    tag="c2", bufs=2)
                                    for ci, (dyi, jj) in enumerate(CH6):
                                        for bk in range(2):
                                            lr = 4 * h + 2 * bk
                                            off = (lr + dyi) * 194 + jj
                                            nc.tensor.matmul(
                                                pc2[:, bk, 0:388],
                                                lhsT=w2p[0][0][:, ci, :],
                                                rhs=slot.span(0, off, 388),
                                                start=(ci == 0), stop=(ci == 5))
                                    # relu+bias into parity scratch
                                    # [64, s(2), 4, 194]; both halves land on
                                    # partitions 0:64 (upper via cross-base ACT)
                                    scr4 = pa.tile([64, 2, 4, 194], f16,
                                                   name="scr4", tag="scr",
                                                   bufs=2)
                                    segs = []
                                    for k in range(4):
                                        i = i0 + k
                                        bc = 1 if i < 4 else (2 if i >= 100 else 0)
                                        if segs and segs[-1][2] == bc:
                                            segs[-1][1] += 1
                                        else:
                                            segs.append([k, 1, bc])
                                    for (k0, kn, bc) in segs:
                                        k = k0
                                        while k < k0 + kn:
                                            if k % 2 == 0 and k0 + kn - k >= 2:
                                                n = (k0 + kn - k) // 2 * 2
                                                srcL = pc2[0:64,
                                                           k // 2:(k + n) // 2,
                                                           0:388]
                                                srcU = pc2[64:128,
                                                           k // 2:(k + n) // 2,
                                                           0:388]
                                            else:
                                                n = 1
                                                cs = (k % 2) * 194
                                                srcL = pc2[0:64, k // 2,
                                                           cs:cs + 194]
                                                srcU = pc2[64:128, k // 2,
                                                           cs:cs + 194]
                                            relu_write(
                                                scr4[:, 0, k:k + n, :], srcL,
                                                bt["b2"][0:64, bc:bc + 1],
                                                use_dve=(h % 2 == 1))
                                            relu_write(
                                                scr4[:, 1, k:k + n, :], srcU,
                                                bt["b2"][0:64, bc:bc + 1],
                                                use_dve=(h % 2 == 0))
                                            k += n
                                    for im_, side in ((3, 0), (100, 1)):
                                        if i0 <= im_ < i0 + 4:
                                            k = im_ - i0
                                            nc.vector.tensor_scalar_mul(
                                                scr4[:, :, k, :],
                                                scr4[:, :, k, :],
                                                rmt[0:64, side:side + 1])
                                    # 2x2 pool: parity max then row-pair max
                                    # (two contiguous TTs beat one strided
                                    # 5D tensor_reduce ~2.5x on DVE)
                                    p0 = 4 * bb + 2 * h
                                    ptmp = pa.tile([64, 4, 192], f16,
                                                   name="ptmp", tag="ptmp",
                                                   bufs=2)
                                    nc.vector.tensor_tensor(
                                        ptmp[:], scr4[:, 0, :, 0:192],
                                        scr4[:, 1, :, 0:192], ALU.max)
                                    nc.vector.tensor_tensor(
                                        pool1.rows(0, p0, 2, 1, 192)[0:64],
                                        ptmp[:, 0:4:2, :], ptmp[:, 1:4:2, :],
                                        ALU.max)
                                    nc.scalar.copy(
                                        pool1.rows(0, p0, 2, 2, 192)[64:128],
                                        pool1.rows(0, p0, 2, 1, 192)[0:64])

                    psum = tc.alloc_tile_pool(name="psum", bufs=8, space="PSUM")
                    # ---- A2: L3, L4, pool2, exchange1 ----
                    if stage == "pool1":
                        nc.sync.dma_start(out=y_out[:], in_=pool1.t[:].bitcast(f32))
                    with tc.tile_pool(name="phA2", bufs=1) as pa2:
                        if SI < 1:
                            raise _StopBuild
                        l3b = Buf(pa2, "l3b", 1, 50, 194)
                        l4b = Buf(pa2, "l4b", 1, 48, 194)
                        l3b.zero_edges(nc)
                        l4b.zero_edges(nc)

                        ch3, grp3 = packed6_chunks()
                        def w3_dst(blk, r0, nrows, pt, ti):
                            for (rs, ln, bc) in seg_rows(r0, nrows, 1, 1, 50):
                                relu_write(
                                    l3b.rows(0, rs, ln, 1, 192),
                                    psum_rows(pt, rs - r0, ln, 1, 192, 194),
                                    bt["b3"][:, bc:bc + 1], use_dve=(ti % 2 == 1))
                        conv_layer(nc, psum, pool1, w3p, ch3, grp3, 1, w3_dst,
                                   [(r, 2) for r in range(0, 50, 2)], "l3")
                        mask_row(l3b, 0, 0)
                        mask_row(l3b, 49, 1)

                        ch4, grp4 = plain_chunks(1)
                        def w4_dst(blk, r0, nrows, pt, ti):
                            relu_write(
                                l4b.rows(0, r0, nrows, 1, 192),
                                psum_rows(pt, 0, nrows, 1, 192, 194),
                                bt["b4"][:, 0:1], use_dve=(ti % 2 == 1))
                        order4 = [0, 1, 2, 21, 22, 23] + list(range(3, 21))
                        conv_layer(nc, psum, l3b, w4p, ch4, grp4, 1, w4_dst,
                                   [(r, 2) for r in range(0, 48, 2)], "l4",
                                   order=order4)

                        if stage == "l4":
                            nc.sync.dma_start(out=y_out[:], in_=l4b.t[:].bitcast(f32))
                        if SI < 2:
                            raise _StopBuild
                        p2tmp = pa2.tile([128, 48, 96], f16, name="p2tmp")
                        def pool2_part(rlo, rhi):
                            n = rhi - rlo
                            nc.vector.tensor_tensor(
                                p2tmp[:, 2 * rlo:2 * rhi, :],
                                l4b.rows(0, 2 * rlo, 2 * n, 1, 192)[:, :, 0:192:2],
                                l4b.rows(0, 2 * rlo, 2 * n, 2, 192)[:, :, 0:192:2],
                                ALU.max)
                            nc.vector.tensor_tensor(
                                pool2.rows(0, 3 + rlo, n, 1, 96),
                                p2tmp[:, 2 * rlo:2 * rhi:2, :],
                                p2tmp[:, 2 * rlo + 1:2 * rhi:2, :], ALU.max)
                        pool2_part(0, 3)
                        pool2_part(21, 24)

                        contrib1 = dram.tile([128, 588], f16, name="contrib1")
                        nc.sync.dma_start(out=contrib1[:, 0:294],
                                          in_=pool2.span(0, 3 * 98, 294))
                        nc.sync.dma_start(out=contrib1[:, 294:588],
                                          in_=pool2.span(0, 24 * 98, 294))
                        ag1 = dram.tile([4, 128, 588], f16, name="ag1")
                        nc.gpsimd.collective_compute(
                            "AllGather", ALU.bypass,
                            replica_groups=[[0, 1, 2, 3], [4, 5, 6, 7]],
                            ins=[contrib1.opt()], outs=[ag1.opt()])
                        pool2_part(3, 21)

                # ======== phase B1 (96-res) ========
                wpB = tc.alloc_tile_pool(name="wpB", bufs=1)
                w8full = wpB.tile([128, 18, 512], f16, name="wts_w8")
                nc.gpsimd.dma_start(out=w8full[:], in_=wp["w8"][:])
                w8p = [(w8full, 0, 18)]
                wpB2 = tc.alloc_tile_pool(name="wpB2", bufs=1)
                w9p = [load_w(wpB2, "w9", 9 * i, 9 * (i + 1), 0)
                       for i in range(4)]
                w10p = [load_w(wpB2, "w10", 9 * i, 9 * (i + 1), 0)
                        for i in range(4)]
                wpB1 = tc.alloc_tile_pool(name="wpB1", bufs=1)
                w5p = [(w5t, 0, 9)]
                w6p = [load_w_static(wpB1, "w6")]
                w7p = [load_w_static(wpB1, "w7")]
                with tc.tile_pool(name="phB1", bufs=1) as pb1:
                    l5b = Buf(pb1, "l5b", 2, 28, 98)
                    l6b = Buf(pb1, "l6b", 2, 26, 98)
                    l7b = Buf(pb1, "l7b", 2, 24, 98)
                    for s in (l5b, l6b, l7b):
                        s.zero_edges(nc)

                    def mk_dst(buf, bkey, mtop, mbot, Rd):
                        W = buf.Wp
                        def f(blk, r0, nrows, pt, ti):
                            for (rs, ln, bc) in seg_rows(r0, nrows, mtop, mbot, Rd):
                                relu_write(
                                    buf.rows(blk, rs, ln, 1, W - 2),
                                    psum_rows(pt, rs - r0, ln, 1, W - 2, W),
                                    bt[bkey][:, 3 * blk + bc:3 * blk + bc + 1],
                                    use_dve=((ti + blk) % 2 == 1))
                        return f

                    def pool3_part(rlo, rhi):
                        n = rhi - rlo
                        p3tmp = pb1.tile([128, 2, 16, 48], f16, name="p3tmp",
                                         tag="p3tmp", bufs=2)
                        for g in range(2):
                            nc.vector.tensor_tensor(
                                p3tmp[:, g, 0:2 * n, :],
                                l7b.rows(g, 2 * rlo, 2 * n, 1, 96)[:, :, 0:96:2],
                                l7b.rows(g, 2 * rlo, 2 * n, 2, 96)[:, :, 0:96:2],
                                ALU.max)
                            nc.vector.tensor_tensor(
                                pool3.rows(g, 3 + rlo, n, 1, 48),
                                p3tmp[:, g, 0:2 * n:2, :],
                                p3tmp[:, g, 1:2 * n:2, :], ALU.max)

                    ch5, grp5 = plain_chunks(1)
                    ch6, grp6 = plain_chunks(2)
                    ch7, grp7 = plain_chunks(2)
                    dst5 = mk_dst(l5b, "b5", 2, 2, 28)
                    dst6 = mk_dst(l6b, "b6", 1, 1, 26)
                    dst7 = mk_dst(l7b, "b7", 0, 0, 24)

                    # --- interior rows: independent of the halo exchange, so
                    # they fill the PE while the AllGathers rendezvous ---
                    with nc.named_scope("B1MID"):
                        if SI >= 3:
                            conv_layer(nc, psum, pool2, w5p, ch5, grp5, 2, dst5,
                                       [(3, 5), (8, 5), (13, 4), (17, 4),
                                        (21, 4)], "l5m")
                        if SI >= 4:
                            conv_layer(nc, psum, l5b, w6p, ch6, grp6, 2, dst6,
                                       [(3, 5), (8, 5), (13, 5), (18, 5)],
                                       "l6m")
                            conv_layer(nc, psum, l6b, w7p, ch7, grp7, 2, dst7,
                                       [(3, 5), (8, 5), (13, 5), (18, 3)],
                                       "l7m")
                        if SI >= 5:
                            pool3_part(2, 10)

                    # --- exchange1 arrives: patch pool2 halo rows ---
                    blocks1 = pb1.tile([128, 4, 588], f16, name="blocks1")
                    for bi in range(4):
                        nc.sync.dma_start(out=blocks1[:, bi, :], in_=ag1[bi])
                    top1 = pool2.span(0, 0, 294)
                    bot1 = pool2.span(0, 27 * 98, 294)
                    # top halo is block q-1 (never 3); bottom is q+1 (never 0)
                    for i in range(3):
                        nc.vector.scalar_tensor_tensor(
                            out=top1, in0=blocks1[:, i, 294:588],
                            scalar=m1t[:, i:i + 1], in1=top1,
                            op0=ALU.mult, op1=ALU.add)
                    for i in range(1, 4):
                        nc.vector.scalar_tensor_tensor(
                            out=bot1, in0=blocks1[:, i, 0:294],
                            scalar=m1t[:, 4 + i:5 + i], in1=bot1,
                            op0=ALU.mult, op1=ALU.add)

                    if stage == "pool2":
                        nc.sync.dma_start(out=y_out[:],
                                          in_=pool2.t[:].bitcast(f32))
                    if SI < 3:
                        raise _StopBuild

                    # --- boundary strips (top/bottom 3 rows per layer) ---
                    with nc.named_scope("B1TB"):
                        conv_layer(nc, psum, pool2, w5p, ch5, grp5, 2, dst5,
                                   [(0, 3)], "l5t")
                        mask_row(l5b, 1, 0)
                        conv_layer(nc, psum, pool2, w5p, ch5, grp5, 2, dst5,
                                   [(25, 3)], "l5u")
                        mask_row(l5b, 26, 1)
                        if stage == "l5":
                            nc.sync.dma_start(out=y_out[:],
                                              in_=l5b.t[:].bitcast(f32))
                        if SI < 4:
                            raise _StopBuild
                        conv_layer(nc, psum, l5b, w6p, ch6, grp6, 2, dst6,
                                   [(0, 3), (23, 3)], "l6t")
                        mask_row(l6b, 0, 0)
                        mask_row(l6b, 25, 1)
                        conv_layer(nc, psum, l6b, w7p, ch7, grp7, 2, dst7,
                                   [(0, 3), (21, 3)], "l7t")
                        if stage == "l7":
                            nc.sync.dma_start(out=y_out[:],
                                              in_=l7b.t[:].bitcast(f32))
                        if SI < 5:
                            raise _StopBuild
                        pool3_part(0, 2)
                        pool3_part(10, 12)

                    contrib2 = dram.tile([128, 600], f16, name="contrib2")
                    for reg, rs in ((0, 3), (1, 12)):
                        for g in range(2):
                            nc.sync.dma_start(
                                out=contrib2[:, reg * 300 + g * 150:
                                             reg * 300 + (g + 1) * 150],
                                in_=pool3.span(g, rs * 50, 150))
                    ag2 = dram.tile([4, 128, 600], f16, name="ag2")
                    nc.gpsimd.collective_compute(
                        "AllGather", ALU.bypass,
                        replica_groups=[[0, 1, 2, 3], [4, 5, 6, 7]],
                        ins=[contrib2.opt()], outs=[ag2.opt()])

                wpB1.release()
                # ======== phase B2 (48-res) ========
                if True:
                    with tc.tile_pool(name="phB2", bufs=1) as pb2:
                        l8b = Buf(pb2, "l8b", 4, 16, 50, extra=512)
                        l9b = Buf(pb2, "l9b", 4, 14, 50, extra=512)
                        outsb = pb2.tile([128, 4, 576], f32, name="outsb")
                        l8b.zero_edges(nc)
                        l9b.zero_edges(nc)

                        def mk_dst2(buf, bkey, mtop, mbot, Rd):
                            def f(blk, r0, nrows, pt, ti):
                                for (rs, ln, bc) in seg_rows(r0, nrows, mtop, mbot, Rd):
                                    relu_write(
                                        buf.rows(blk, rs, ln, 1, 48),
                                        psum_rows(pt, rs - r0, ln, 1, 48, 50),
                                        bt[bkey][:, 3 * blk + bc:3 * blk + bc + 1],
                                        use_dve=((ti + blk) % 2 == 1))
                            return f

                        dst8 = mk_dst2(l8b, "b8", 2, 2, 16)
                        dst9 = mk_dst2(l9b, "b9", 1, 1, 14)
                        ch8, grp8 = plain_chunks(2)
                        ch9, grp9 = plain_chunks(4)
                        ch10, grp10 = plain_chunks(4)

                        def w10_dst(blk, r0, nrows, pt, ti):
                            relu_write(
                                outsb[:, blk, r0 * 48:(r0 + nrows) * 48]
                                .rearrange("p (r w) -> p r w", r=nrows, w=48),
                                psum_rows(pt, 0, nrows, 1, 48, 50),
                                bt["b10"][:, 3 * blk:3 * blk + 1],
                                use_dve=((ti + blk) % 2 == 1))

                        yr = y_out.rearrange("b p n -> p b n")
                        # --- rows needing only exchange1: run during the
                        # exchange2 rendezvous ---
                        if SI >= 6:
                            with nc.named_scope("B2C1"):
                                conv_layer_cm(nc, psum, pool3, w8p, ch8, grp8,
                                              4, dst8, [(3, 10)], "l8a")
                                conv_layer_cm(nc, psum, l8b, w9p, ch9, grp9,
                                              4, dst9, [(3, 8)], "l9a")
                                conv_layer_cm(nc, psum, l9b, w10p, ch10,
                                              grp10, 4, w10_dst, [(3, 6)],
                                              "l10a")
                            for blk in range(4):
                                nc.sync.dma_start(
                                    out=yr[:, blk, 3 * 48:9 * 48],
                                    in_=outsb[:, blk, 3 * 48:9 * 48])

                        # --- exchange2 arrives: patch pool3 halo rows ---
                        blocks2 = pb2.tile([128, 4, 600], f16, name="blocks2")
                        for bi in range(4):
                            nc.sync.dma_start(out=blocks2[:, bi, :],
                                              in_=ag2[bi])
                        for g in range(2):
                            top2 = pool3.span(g, 0, 150)
                            bot2 = pool3.span(g, 15 * 50, 150)
                            for i in range(3):
                                nc.vector.scalar_tensor_tensor(
                                    out=top2,
                                    in0=blocks2[:, i, 300 + g * 150:
                                                300 + (g + 1) * 150],
                                    scalar=m2t[:, i:i + 1], in1=top2,
                                    op0=ALU.mult, op1=ALU.add)
                            for i in range(1, 4):
                                nc.vector.scalar_tensor_tensor(
                                    out=bot2,
                                    in0=blocks2[:, i, g * 150:(g + 1) * 150],
                                    scalar=m2t[:, 4 + i:5 + i], in1=bot2,
                                    op0=ALU.mult, op1=ALU.add)

                        if stage == "pool3":
                            nc.sync.dma_start(
                                out=y_out[:],
                                in_=pool3.t[:, 0:2 * (18 * 50 + 5)]
                                .bitcast(f32))
                        if SI < 6:
                            raise _StopBuild

                        # --- boundary strips after exchange2 ---
                        with nc.named_scope("B2C2"):
                            conv_layer_cm(nc, psum, pool3, w8p, ch8, grp8, 4,
                                          dst8, [[(0, 3), (13, 3)]], "l8z")
                            mask_row(l8b, 1, 0)
                            mask_row(l8b, 14, 1)
                            conv_layer_cm(nc, psum, l8b, w9p, ch9, grp9, 4,
                                          dst9, [[(0, 3), (11, 3)]], "l9z")
                            mask_row(l9b, 0, 0)
                            mask_row(l9b, 13, 1)
                            if stage == "l9":
                                nc.sync.dma_start(
                                    out=y_out[:],
                                    in_=l9b.t[:, 0:4 * (14 * 50 + 5)]
                                    .bitcast(f32))
                            if SI < 7:
                                raise _StopBuild
                            conv_layer_cm(nc, psum, l9b, w10p, ch10, grp10,
                                          4, w10_dst, [[(0, 3), (9, 3)]],
                                          "l10z")

                        for blk in range(4):
                            nc.sync.dma_start(out=yr[:, blk, 0:3 * 48],
                                              in_=outsb[:, blk, 0:3 * 48])
                            nc.sync.dma_start(out=yr[:, blk, 9 * 48:576],
                                              in_=outsb[:, blk, 9 * 48:576])

                wpB2.release()
                wpB.release()
            except _StopBuild:
                pass
            finally:
                if psum is not None:
                    psum.release()
    nc.finalize()
    return nc


# ---------------------------------------------------------------------------
# host side
# ---------------------------------------------------------------------------
def _pack_all_weights(kw):
    w = {}
    # conv1 pair-packed: K=(dy,et,c), M=[64 odd y1[2t-1] | 64 even y1[2t]]
    a = np.zeros((36, 1, 128), np.float32)
    w1r = kw["w1"]  # [64, 3, 3, 3]
    for dyi in range(3):
        for et in (-1, 0, 1, 2):
            for c in range(3):
                k = 12 * dyi + 3 * (et + 1) + c
                if -1 <= et <= 1:  # odd half: dx index et+1
                    a[k, 0, 0:64] = w1r[:, c, dyi, et + 1]
                if 0 <= et <= 2:   # even half: dx index et
                    a[k, 0, 64:128] = w1r[:, c, dyi, et]
    w["w1"] = a

    def pack6(src, dup_m):
        O = src.shape[0]
        w9 = src.reshape(O, 64, 3, 3)
        M = 2 * O if dup_m else O
        out = np.zeros((128, 6, M), np.float32)
        for d in range(3):
            av = np.zeros((128, O), np.float32)
            av[0:64] = w9[:, :, d, 1].T
            av[64:128] = w9[:, :, d, 0].T
            bv = np.zeros((128, O), np.float32)
            bv[0:64] = w9[:, :, d, 2].T
            if dup_m:
                av = np.concatenate([av, av], axis=1)
                bv = np.concatenate([bv, bv], axis=1)
            out[:, 2 * d] = av
            out[:, 2 * d + 1] = bv
        return np.ascontiguousarray(out)

    # conv2 pair-packed: K=[64 odd-in | 64 even-in], M=[64 even-out | 64 odd-out]
    w2r = kw["w2"]  # [64, 64, 3, 3]
    b2 = np.zeros((128, 6, 128), np.float32)
    for dyi in range(3):
        for jj in range(2):
            ci = 2 * dyi + jj
            b2[0:64, ci, 0:64] = w2r[:, :, dyi, 2 * jj].T
            if jj == 1:
                b2[0:64, ci, 64:128] = w2r[:, :, dyi, 1].T
            if jj == 0:
                b2[64:128, ci, 0:64] = w2r[:, :, dyi, 1].T
            b2[64:128, ci, 64:128] = w2r[:, :, dyi, 2 * jj].T
    w["w2"] = np.ascontiguousarray(b2)
    w["w3"] = pack6(kw["w3"], False)

    def packplain(src, cin_g):
        O, I = src.shape[0], src.shape[1]
        s9 = src.reshape(O, I, 9)
        arr = np.zeros((128, cin_g * 9, O), np.float32)
        for g in range(cin_g):
            for t in range(9):
                arr[:, g * 9 + t, :] = s9[:, g * 128:(g + 1) * 128, t].T
        return np.ascontiguousarray(arr)

    w["w4"] = packplain(kw["w4"], 1)
    w["w5"] = packplain(kw["w5"], 1)
    w["w6"] = packplain(kw["w6"], 2)
    w["w7"] = packplain(kw["w7"], 2)
    w["w8"] = packplain(kw["w8"], 2)
    w["w9"] = packplain(kw["w9"], 4)
    w["w10"] = packplain(kw["w10"], 4)
    return w


def _host_inputs(batch, weights, kb):
    maps = []
    NBLK = {"b1": 1, "b2": 1, "b3": 1, "b4": 1, "b5": 2, "b6": 2, "b7": 2,
            "b8": 4, "b9": 4, "b10": 4}
    for core in range(N_CORES):
        b, q = divmod(core, 4)
        x = np.zeros((3, 110, 386), np.float32)
        glo, ghi = 96 * q - 7, 96 * q + 103
        vlo, vhi = max(0, glo), min(384, ghi)
        x[:, vlo - glo:vhi - glo, 1:385] = batch[b, :, vlo:vhi, :]
        xf = np.zeros((3, 110 * 386 + 2), np.float32)
        xf[:, 1:1 + 110 * 386] = x.reshape(3, -1)
        # im2col on host: partition (dy, et, c) holds xf[c] shifted so the
        # kernel's superblock DMA is one contiguous [36, n] transfer.
        im2 = np.zeros((36, 43624), np.float32)
        for dyi, dy in enumerate((-1, 0, 1)):
            for ei, et in enumerate((-1, 0, 1, 2)):
                for c in range(3):
                    p = 12 * dyi + 3 * ei + c
                    base = 1 + (2 + dy) * 386 + et
                    seg = xf[c, base:]
                    im2[p, :seg.shape[0]] = seg

        m = dict(x=im2.astype(np.float16))
        m.update(weights)
        for name, nblk in NBLK.items():
            bv = kb[name]
            arr = np.zeros((128, nblk * 3), np.float32)
            for blk in range(nblk):
                col = np.concatenate([bv, bv]) if bv.shape[0] == 64 \
                    else bv[blk * 128:(blk + 1) * 128]
                arr[:, 3 * blk] = col
                arr[:, 3 * blk + 1] = 0.0 if q == 0 else col
                arr[:, 3 * blk + 2] = 0.0 if q == 3 else col
            m[name] = arr

        top = np.zeros(4, np.float32)
        bot = np.zeros(4, np.float32)
        if q > 0:
            top[q - 1] = 1
        if q < 3:
            bot[q + 1] = 1
        mk = np.tile(np.concatenate([top, bot])[None, :], (128, 1)).astype(np.float32)
        m["masks1"] = mk
        m["masks2"] = mk.copy()
        rm = np.ones((128, 2), np.float32)
        if q == 0:
            rm[:, 0] = 0.0
        if q == 3:
            rm[:, 1] = 0.0
        m["rowmask"] = rm
        maps.append(m)
    return maps


_NC_CACHE = {}


def kernel(batch, pooling_mask, w1, b1, w2, b2, w3, b3, w4, b4, w5, b5,
           w6, b6, w7, b7, w8, b8, w9, b9, w10, b10, _trace=False,
           _trace_cores=None):
    batch = np.asarray(batch, np.float32)
    kw = {"w1": w1, "w2": w2, "w3": w3, "w4": w4, "w5": w5, "w6": w6,
          "w7": w7, "w8": w8, "w9": w9, "w10": w10}
    kw = {k: np.asarray(v, np.float32) for k, v in kw.items()}
    kb = {"b1": b1, "b2": b2, "b3": b3, "b4": b4, "b5": b5, "b6": b6,
          "b7": b7, "b8": b8, "b9": b9, "b10": b10}
    kb = {k: np.asarray(v, np.float32) for k, v in kb.items()}

    if "nc" not in _NC_CACHE:
        _NC_CACHE["nc"] = build_nc()
    nc = _NC_CACHE["nc"]

    weights = {k: v.astype(np.float16)
               for k, v in _pack_all_weights(kw).items()}
    in_maps = _host_inputs(batch, weights, kb)
    res = run_bass_kernel_spmd(nc, in_maps, core_ids=list(range(N_CORES)),
                               trace=_trace, trace_cores=_trace_cores)

    out = np.zeros((2, 512, 48, 48), np.float32)
    for core in range(N_CORES):
        b, q = divmod(core, 4)
        o = np.asarray(res.results[core]["out"])
        o = o.reshape(4, 128, 12, 48).reshape(512, 12, 48)
        out[b, :, 12 * q:12 * (q + 1), :] = o
    if _trace:
        return out, res
    return out

